# revision 8
# baseline (speedup 1.0000x reference)
"""Trainium2 Bass kernel for a 3-layer MLP classifier.

  x:[16,512,256,5,5] -> rows [8192, 6400]
  out = relu(relu(x@W1+b1)@W2+b2)@W3+b3 -> [16, 512, 21]

Data-parallel over 8 NeuronCores: 1024 rows/core, weights replicated.

Per-core pipeline (L1/L2/L3 matmuls run in float32r for full PE rate; walrus
requires fp32r matmul inputs to be produced by fp32r-rounding ops or DMA,
N/K multiples of 32, and 32-aligned partition starts):
  - x rows DMA'd naturally as [128 rows, 3200] f32 chunks.
  - PE transposes 128x128 f32 tiles of x into PSUM; DVE copies PSUM->SBUF
    casting to f32r (the rounding producer).
  - L1: psum_h1T[oi] += W1_lhsT @ xT -> h1^T [256 ch, 512 rows]; channel on
    partitions so relu+b1 is one per-partition ScalarE activation -> f32r.
  - L2: lhsT=W2 chunk, rhs=h1^T -> h2^T [64, 512]; relu+b2 likewise.
  - L3: lhsT = h2^T slice padded to K=96 (row 64 = ones for the b3 trick,
    rows 65:96 zeros), rhs = W3ext [96, 32] (b3 in row 64, zero-padded) ->
    natural-orientation out [128 rows, 32] in PSUM; DVE copies the [*,0:21]
    columns to SBUF; DMA to DRAM.
"""

from contextlib import ExitStack

import numpy as np

import concourse.bass as bass
import concourse.mybir as mybir
import concourse.tile as tile
from concourse import bacc
from concourse.bass_utils import run_bass_kernel_spmd
from concourse.masks import make_identity

F32 = mybir.dt.float32
F32R = mybir.dt.float32r
RELU = mybir.ActivationFunctionType.Relu
IDENT = mybir.ActivationFunctionType.Identity

N_CORES = 8
ROWS_TOTAL = 16 * 512            # 8192
ROWS = ROWS_TOTAL // N_CORES     # 1024 rows per core
D_IN = 6400                      # 256 * 5 * 5
H1 = 256
H2 = 64
N_CLS = 21
N_PAD = 32                       # L3 moving dim padded (mult of 32)
K3 = 96                          # L3 contraction padded (64 + ones + zeros)

BLK = 512                        # rows per compute block (PSUM bank = 512 f32)
RSUB = BLK // 128                # 4 row sub-tiles per block
N_BLK = ROWS // BLK              # 2 blocks per core
KI = D_IN // 128                 # 50 contraction chunks
DC = 2                           # x column-chunks per row sub-tile
DCW = D_IN // DC                 # 3200 elements per chunk (1.64MB DMA per tile)
KI_PER_DC = DCW // 128           # 25


def build_program():
    nc = bacc.Bacc("TRN2", target_bir_lowering=False, debug=False)

    x_d = nc.dram_tensor("x", [ROWS, D_IN], F32, kind="ExternalInput").ap()
    w1_d = nc.dram_tensor("W1", [D_IN, H1], F32, kind="ExternalInput").ap()
    b1_d = nc.dram_tensor("b1", [H1], F32, kind="ExternalInput").ap()
    w2_d = nc.dram_tensor("W2", [H1, H2], F32, kind="ExternalInput").ap()
    b2_d = nc.dram_tensor("b2", [H2], F32, kind="ExternalInput").ap()
    w3_d = nc.dram_tensor("W3", [H2, N_CLS], F32, kind="ExternalInput").ap()
    b3_d = nc.dram_tensor("b3", [N_CLS], F32, kind="ExternalInput").ap()
    out_d = nc.dram_tensor("out", [ROWS, N_CLS], F32, kind="ExternalOutput").ap()

    with tile.TileContext(nc) as tc, ExitStack() as ctx:
        const = ctx.enter_context(tc.tile_pool(name="const", bufs=1))
        xnat_p = ctx.enter_context(tc.tile_pool(name="xnat", bufs=8))
        xt_p = ctx.enter_context(tc.tile_pool(name="xt", bufs=4))
        h_p = ctx.enter_context(tc.tile_pool(name="h", bufs=4))
        o_p = ctx.enter_context(tc.tile_pool(name="o", bufs=2))
        ptp_p = ctx.enter_context(tc.tile_pool(name="ptp", bufs=3, space="PSUM"))
        ph1_p = ctx.enter_context(tc.tile_pool(name="ph1", bufs=2, space="PSUM"))
        ph2_p = ctx.enter_context(tc.tile_pool(name="ph2", bufs=1, space="PSUM"))
        po_p = ctx.enter_context(tc.tile_pool(name="po", bufs=1, space="PSUM"))

        # ---- constants / weights (loaded once) ----
        ident = const.tile([128, 128], F32)
        make_identity(nc, ident[:])

        # W1 as lhsT tiles: w1_sb[p, ki, o] = W1[ki*128 + p, o]
        w1_sb = const.tile([128, KI, H1], F32R)
        nc.sync.dma_start(
            w1_sb[:], w1_d.rearrange("(ki p) o -> p ki o", p=128).bitcast(F32R)
        )

        # W2 as lhsT tiles: w2_sb[p, ci, o] = W2[ci*128 + p, o]
        w2_sb = const.tile([128, H1 // 128, H2], F32R)
        nc.sync.dma_start(
            w2_sb[:], w2_d.rearrange("(ci p) o -> p ci o", p=128).bitcast(F32R)
        )

        # W3 extended [96, 32]: [0:64, 0:21] = W3, row 64 = b3, rest zeros
        w3x_sb = const.tile([K3, N_PAD], F32R)
        nc.sync.dma_start(w3x_sb[:H2, :N_CLS], w3_d.bitcast(F32R))
        nc.scalar.activation(
            w3x_sb[:H2, N_CLS:], ident[:H2, : N_PAD - N_CLS],
            IDENT, bias=0.0, scale=0.0,
        )
        nc.scalar.activation(
            w3x_sb[H2:K3, :], ident[: K3 - H2, :N_PAD],
            IDENT, bias=0.0, scale=0.0,
        )
        nc.sync.dma_start(
            w3x_sb[H2 : H2 + 1, :N_CLS],
            b3_d.rearrange("(a c) -> a c", a=1).bitcast(F32R),
        )

        # biases as per-partition columns (ACT bias inputs, f32)
        b1_sb = const.tile([128, H1 // 128], F32)
        nc.sync.dma_start(b1_sb[:], b1_d.rearrange("(oi p) -> p oi", p=128))
        b2_sb = const.tile([H2, 1], F32)
        nc.sync.dma_start(b2_sb[:], b2_d.rearrange("(c a) -> c a", a=1))

        # ---- main loop over row blocks ----
        for blk in range(N_BLK):
            r0 = blk * BLK

            # stream x naturally: [128 rows, DCW] f32 chunks, (dc, rs) order
            xn = []
            for dc in range(DC):
                row = []
                for rs in range(RSUB):
                    t = xnat_p.tile([128, DCW], F32, tag="xn", bufs=8)
                    nc.sync.dma_start(
                        t[:],
                        x_d[
                            r0 + rs * 128 : r0 + (rs + 1) * 128,
                            dc * DCW : (dc + 1) * DCW,
                        ],
                    )
                    row.append(t)
                xn.append(row)

            ph1 = []
            for oi in range(H1 // 128):
                pt = ph1_p.tile([128, BLK], F32, tag="ph1", bufs=2)
                ph1.append(pt)

            for dc in range(DC):
                for kl in range(KI_PER_DC):
                    ki = dc * KI_PER_DC + kl
                    ptp = ptp_p.tile([128, BLK], F32, tag="ptp", bufs=3)
                    for rs in range(RSUB):
                        nc.tensor.transpose(
                            ptp[:, rs * 128 : (rs + 1) * 128],
                            xn[dc][rs][:, kl * 128 : (kl + 1) * 128],
                            ident[:],
                        )
                    # rounding cast f32 -> f32r while evacuating PSUM
                    xt = xt_p.tile([128, BLK], F32R, tag="xt", bufs=4)
                    nc.vector.tensor_copy(xt[:], ptp[:])
                    for oi in range(H1 // 128):
                        nc.tensor.matmul(
                            ph1[oi][:],
                            w1_sb[:, ki, oi * 128 : (oi + 1) * 128],
                            xt[:],
                            start=(ki == 0),
                            stop=(ki == KI - 1),
                        )

            # h1^T = relu(psum + b1): [256, 512] as two f32r tiles
            h1t = []
            for oi in range(H1 // 128):
                ht = h_p.tile([128, BLK], F32R, tag="h1t", bufs=4)
                nc.scalar.activation(
                    ht[:], ph1[oi][:], RELU, bias=b1_sb[:, oi : oi + 1]
                )
                h1t.append(ht)

            # L2 -> h2^T [64, 512] (+ padding rows for the L3 lhsT)
            ph2 = ph2_p.tile([H2, BLK], F32, tag="ph2", bufs=1)
            for ci in range(H1 // 128):
                nc.tensor.matmul(
                    ph2[:],
                    w2_sb[:, ci, :],
                    h1t[ci][:],
                    start=(ci == 0),
                    stop=(ci == H1 // 128 - 1),
                )
            h2t = h_p.tile([K3, BLK], F32R, tag="h2t", bufs=2)
            nc.scalar.activation(h2t[:H2, :], ph2[:], RELU, bias=b2_sb[:])
            # rows 64:96 zeros, then row 64 = ones (b3 trick)
            nc.scalar.activation(
                h2t[H2:K3, :], ph2[: K3 - H2, :], IDENT, bias=0.0, scale=0.0
            )
            nc.scalar.activation(
                h2t[H2 : H2 + 1, :], ph2[0:1, :], IDENT, bias=1.0, scale=0.0
            )

            # L3: natural-orientation output [128 rows, 32] per sub-tile
            po = po_p.tile([128, RSUB * N_PAD], F32, tag="po", bufs=1)
            for rs in range(RSUB):
                nc.tensor.matmul(
                    po[:, rs * N_PAD : (rs + 1) * N_PAD],
                    h2t[:, rs * 128 : (rs + 1) * 128],
                    w3x_sb[:],
                    start=True,
                    stop=True,
                )
            ot = o_p.tile([128, RSUB * N_CLS], F32, tag="ot", bufs=2)
            nc.vector.tensor_copy(
                ot[:].rearrange("p (rs c) -> p rs c", c=N_CLS),
                po[:].rearrange("p (rs c) -> p rs c", c=N_PAD)[:, :, :N_CLS],
            )
            nc.sync.dma_start(
                out_d[r0 : r0 + BLK, :].rearrange("(rs p) c -> p rs c", p=128),
                ot[:].rearrange("p (rs c) -> p rs c", c=N_CLS),
            )

    nc.compile()
    return nc


_NC_CACHE = None


def kernel(**inputs) -> np.ndarray:
    global _NC_CACHE
    if _NC_CACHE is None:
        _NC_CACHE = build_program()
    nc = _NC_CACHE

    x = np.ascontiguousarray(inputs["x"], dtype=np.float32).reshape(ROWS_TOTAL, D_IN)
    common = {
        "W1": np.ascontiguousarray(inputs["W1"], dtype=np.float32),
        "b1": np.ascontiguousarray(inputs["b1"], dtype=np.float32),
        "W2": np.ascontiguousarray(inputs["W2"], dtype=np.float32),
        "b2": np.ascontiguousarray(inputs["b2"], dtype=np.float32),
        "W3": np.ascontiguousarray(inputs["W3"], dtype=np.float32),
        "b3": np.ascontiguousarray(inputs["b3"], dtype=np.float32),
    }
    in_maps = [
        {"x": x[i * ROWS : (i + 1) * ROWS], **common} for i in range(N_CORES)
    ]
    res = run_bass_kernel_spmd(nc, in_maps, list(range(N_CORES)))
    out = np.concatenate([res.results[i]["out"] for i in range(N_CORES)], axis=0)
    return out.reshape(16, 512, N_CLS).astype(np.float32)


# revision 10
# speedup vs baseline: 73.4911x; 73.4911x over previous
"""Trainium2 Bass kernel for a 3-layer MLP classifier.

  x:[16,512,256,5,5] -> rows [8192, 6400]
  out = relu(relu(x@W1+b1)@W2+b2)@W3+b3 -> [16, 512, 21]

Data-parallel over 8 NeuronCores: 1024 rows/core, weights replicated.

Per-core pipeline (L1/L2/L3 matmuls run in float32r for full PE rate; walrus
requires fp32r matmul inputs to be produced by fp32r-rounding ops or DMA,
N/K multiples of 32, and 32-aligned partition starts):
  - x rows DMA'd naturally as [128 rows, 3200] f32 chunks.
  - PE transposes 128x128 f32 tiles of x into PSUM; DVE copies PSUM->SBUF
    casting to f32r (the rounding producer).
  - L1: psum_h1T[oi] += W1_lhsT @ xT -> h1^T [256 ch, 512 rows]; channel on
    partitions so relu+b1 is one per-partition ScalarE activation -> f32r.
  - L2: lhsT=W2 chunk, rhs=h1^T -> h2^T [64, 512]; relu+b2 likewise.
  - L3: lhsT = h2^T slice padded to K=96 (row 64 = ones for the b3 trick,
    rows 65:96 zeros), rhs = W3ext [96, 32] (b3 in row 64, zero-padded) ->
    natural-orientation out [128 rows, 32] in PSUM; DVE copies the [*,0:21]
    columns to SBUF; DMA to DRAM.
"""

from contextlib import ExitStack

import numpy as np

import concourse.bass as bass
import concourse.mybir as mybir
import concourse.tile as tile
from concourse import bacc
from concourse.bass_utils import run_bass_kernel_spmd
from concourse.masks import make_identity

F32 = mybir.dt.float32
F32R = mybir.dt.float32r
RELU = mybir.ActivationFunctionType.Relu
IDENT = mybir.ActivationFunctionType.Identity

N_CORES = 8
ROWS_TOTAL = 16 * 512            # 8192
ROWS = ROWS_TOTAL // N_CORES     # 1024 rows per core
D_IN = 6400                      # 256 * 5 * 5
H1 = 256
H2 = 64
N_CLS = 21
N_PAD = 32                       # L3 moving dim padded (mult of 32)
K3 = 96                          # L3 contraction padded (64 + ones + zeros)

BLK = 512                        # rows per compute block (PSUM bank = 512 f32)
RSUB = BLK // 128                # 4 row sub-tiles per block
N_BLK = ROWS // BLK              # 2 blocks per core
KI = D_IN // 128                 # 50 contraction chunks
DC = 2                           # x column-chunks per row sub-tile
DCW = D_IN // DC                 # 3200 elements per chunk (1.64MB DMA per tile)
KI_PER_DC = DCW // 128           # 25


def build_program(repeat: int = 1):
    nc = bacc.Bacc("TRN2", target_bir_lowering=False, debug=False)

    x_d = nc.dram_tensor("x", [ROWS, D_IN], F32, kind="ExternalInput").ap()
    w1_d = nc.dram_tensor("W1", [D_IN, H1], F32, kind="ExternalInput").ap()
    b1_d = nc.dram_tensor("b1", [H1], F32, kind="ExternalInput").ap()
    w2_d = nc.dram_tensor("W2", [H1, H2], F32, kind="ExternalInput").ap()
    b2_d = nc.dram_tensor("b2", [H2], F32, kind="ExternalInput").ap()
    w3_d = nc.dram_tensor("W3", [H2, N_CLS], F32, kind="ExternalInput").ap()
    b3_d = nc.dram_tensor("b3", [N_CLS], F32, kind="ExternalInput").ap()
    out_d = nc.dram_tensor("out", [ROWS, N_CLS], F32, kind="ExternalOutput").ap()

    with tile.TileContext(nc) as tc, ExitStack() as ctx:
        const = ctx.enter_context(tc.tile_pool(name="const", bufs=1))
        xnat_p = ctx.enter_context(tc.tile_pool(name="xnat", bufs=8))
        xt_p = ctx.enter_context(tc.tile_pool(name="xt", bufs=4))
        h_p = ctx.enter_context(tc.tile_pool(name="h", bufs=4))
        o_p = ctx.enter_context(tc.tile_pool(name="o", bufs=2))
        ptp_p = ctx.enter_context(tc.tile_pool(name="ptp", bufs=3, space="PSUM"))
        ph1_p = ctx.enter_context(tc.tile_pool(name="ph1", bufs=2, space="PSUM"))
        ph2_p = ctx.enter_context(tc.tile_pool(name="ph2", bufs=1, space="PSUM"))
        po_p = ctx.enter_context(tc.tile_pool(name="po", bufs=1, space="PSUM"))

        # ---- constants / weights (loaded once) ----
        ident = const.tile([128, 128], F32)
        make_identity(nc, ident[:])

        # W1 as lhsT tiles: w1_sb[p, ki, o] = W1[ki*128 + p, o]
        w1_sb = const.tile([128, KI, H1], F32R)
        nc.sync.dma_start(
            w1_sb[:], w1_d.rearrange("(ki p) o -> p ki o", p=128).bitcast(F32R)
        )

        # W2 as lhsT tiles: w2_sb[p, ci, o] = W2[ci*128 + p, o]
        w2_sb = const.tile([128, H1 // 128, H2], F32R)
        nc.sync.dma_start(
            w2_sb[:], w2_d.rearrange("(ci p) o -> p ci o", p=128).bitcast(F32R)
        )

        # W3 extended [96, 32]: [0:64, 0:21] = W3, row 64 = b3, rest zeros
        w3x_sb = const.tile([K3, N_PAD], F32R)
        nc.sync.dma_start(w3x_sb[:H2, :N_CLS], w3_d.bitcast(F32R))
        nc.scalar.activation(
            w3x_sb[:H2, N_CLS:], ident[:H2, : N_PAD - N_CLS],
            IDENT, bias=0.0, scale=0.0,
        )
        nc.scalar.activation(
            w3x_sb[H2:K3, :], ident[: K3 - H2, :N_PAD],
            IDENT, bias=0.0, scale=0.0,
        )
        nc.sync.dma_start(
            w3x_sb[H2 : H2 + 1, :N_CLS],
            b3_d.rearrange("(a c) -> a c", a=1).bitcast(F32R),
        )

        # biases as per-partition columns (ACT bias inputs, f32)
        b1_sb = const.tile([128, H1 // 128], F32)
        nc.sync.dma_start(b1_sb[:], b1_d.rearrange("(oi p) -> p oi", p=128))
        b2_sb = const.tile([H2, 1], F32)
        nc.sync.dma_start(b2_sb[:], b2_d.rearrange("(c a) -> c a", a=1))

        # ---- main loop over row blocks ----
        for blk in range(N_BLK * repeat):
            r0 = (blk % N_BLK) * BLK

            # stream x naturally: [128 rows, DCW] f32 chunks, (dc, rs) order
            xn = []
            for dc in range(DC):
                row = []
                for rs in range(RSUB):
                    t = xnat_p.tile([128, DCW], F32, tag="xn", bufs=8)
                    nc.sync.dma_start(
                        t[:],
                        x_d[
                            r0 + rs * 128 : r0 + (rs + 1) * 128,
                            dc * DCW : (dc + 1) * DCW,
                        ],
                    )
                    row.append(t)
                xn.append(row)

            ph1 = []
            for oi in range(H1 // 128):
                pt = ph1_p.tile([128, BLK], F32, tag="ph1", bufs=2)
                ph1.append(pt)

            for dc in range(DC):
                for kl in range(KI_PER_DC):
                    ki = dc * KI_PER_DC + kl
                    ptp = ptp_p.tile([128, BLK], F32, tag="ptp", bufs=3)
                    for rs in range(RSUB):
                        nc.tensor.transpose(
                            ptp[:, rs * 128 : (rs + 1) * 128],
                            xn[dc][rs][:, kl * 128 : (kl + 1) * 128],
                            ident[:],
                        )
                    # rounding cast f32 -> f32r while evacuating PSUM
                    xt = xt_p.tile([128, BLK], F32R, tag="xt", bufs=4)
                    nc.vector.tensor_copy(xt[:], ptp[:])
                    for oi in range(H1 // 128):
                        nc.tensor.matmul(
                            ph1[oi][:],
                            w1_sb[:, ki, oi * 128 : (oi + 1) * 128],
                            xt[:],
                            start=(ki == 0),
                            stop=(ki == KI - 1),
                        )

            # h1^T = relu(psum + b1): [256, 512] as two f32r tiles
            h1t = []
            for oi in range(H1 // 128):
                ht = h_p.tile([128, BLK], F32R, tag="h1t", bufs=4)
                nc.scalar.activation(
                    ht[:], ph1[oi][:], RELU, bias=b1_sb[:, oi : oi + 1]
                )
                h1t.append(ht)

            # L2 -> h2^T [64, 512] (+ padding rows for the L3 lhsT)
            ph2 = ph2_p.tile([H2, BLK], F32, tag="ph2", bufs=1)
            for ci in range(H1 // 128):
                nc.tensor.matmul(
                    ph2[:],
                    w2_sb[:, ci, :],
                    h1t[ci][:],
                    start=(ci == 0),
                    stop=(ci == H1 // 128 - 1),
                )
            h2t = h_p.tile([K3, BLK], F32R, tag="h2t", bufs=2)
            nc.scalar.activation(h2t[:H2, :], ph2[:], RELU, bias=b2_sb[:])
            # rows 64:96 zeros, then row 64 = ones (b3 trick)
            nc.scalar.activation(
                h2t[H2:K3, :], ph2[: K3 - H2, :], IDENT, bias=0.0, scale=0.0
            )
            nc.scalar.activation(
                h2t[H2 : H2 + 1, :], ph2[0:1, :], IDENT, bias=1.0, scale=0.0
            )

            # L3: natural-orientation output [128 rows, 32] per sub-tile
            po = po_p.tile([128, RSUB * N_PAD], F32, tag="po", bufs=1)
            for rs in range(RSUB):
                nc.tensor.matmul(
                    po[:, rs * N_PAD : (rs + 1) * N_PAD],
                    h2t[:, rs * 128 : (rs + 1) * 128],
                    w3x_sb[:],
                    start=True,
                    stop=True,
                )
            ot = o_p.tile([128, RSUB * N_CLS], F32, tag="ot", bufs=2)
            nc.vector.tensor_copy(
                ot[:].rearrange("p (rs c) -> p rs c", c=N_CLS),
                po[:].rearrange("p (rs c) -> p rs c", c=N_PAD)[:, :, :N_CLS],
            )
            nc.sync.dma_start(
                out_d[r0 : r0 + BLK, :].rearrange("(rs p) c -> p rs c", p=128),
                ot[:].rearrange("p (rs c) -> p rs c", c=N_CLS),
            )

    nc.compile()
    return nc


_NC_CACHE = None


def kernel(**inputs) -> np.ndarray:
    global _NC_CACHE
    if _NC_CACHE is None:
        _NC_CACHE = build_program()
    nc = _NC_CACHE

    x = np.ascontiguousarray(inputs["x"], dtype=np.float32).reshape(ROWS_TOTAL, D_IN)
    common = {
        "W1": np.ascontiguousarray(inputs["W1"], dtype=np.float32),
        "b1": np.ascontiguousarray(inputs["b1"], dtype=np.float32),
        "W2": np.ascontiguousarray(inputs["W2"], dtype=np.float32),
        "b2": np.ascontiguousarray(inputs["b2"], dtype=np.float32),
        "W3": np.ascontiguousarray(inputs["W3"], dtype=np.float32),
        "b3": np.ascontiguousarray(inputs["b3"], dtype=np.float32),
    }
    in_maps = [
        {"x": x[i * ROWS : (i + 1) * ROWS], **common} for i in range(N_CORES)
    ]
    res = run_bass_kernel_spmd(nc, in_maps, list(range(N_CORES)))
    out = np.concatenate([res.results[i]["out"] for i in range(N_CORES)], axis=0)
    return out.reshape(16, 512, N_CLS).astype(np.float32)
